# revision 11
# baseline (speedup 1.0000x reference)
"""Trainium2 Bass kernel: batched ChebConv GNN with L1-distance adjacency.

Pipeline per sample (N=512 nodes, F=625 features):
  1. Sort nodes by attention (host). All pairs with |att_i-att_j| <= 0.05
     then lie within a rank band |i-j| <= w (w computed exactly on host).
  2. Banded pairwise L1 distances on device using the exact identity
     sum_f |a-b| = 2*sum_f max(a,b) - S_i - S_j  (S = row sums), computed
     with one fused DVE op per (offset, feature-chunk) + a PE ones-matmul
     for the cross-partition feature reduction.
  3. Threshold masks -> banded adjacency -> scattered to a DRAM matrix via
     diagonal-stride DMAs (identity on the diagonal).
  4. Degree-normalized ChebConv x2 as float16 PE matmuls in transposed
     layouts (the dinv column scaling is commuted through the weight
     matmuls so it is always a cheap per-partition row scaling). The
     adjacency matmuls (psZ/psQ) are restricted to the band segments.
Data parallel over batch: 16 samples, 8 cores, 2 samples/core.

dtype strategy (KERNEL_MM_DTYPE=f16 default): fp16 gives the DVE max ops
2x mode (2 elem/cycle/lane), halves DMA bytes, and keeps PE matmuls at
1 cycle/row even for small free sizes. max(a,b) of fp16 inputs is exact;
distance error comes only from rounding x to fp16 (~3e-3 absolute on
D ~ 176), so threshold misclassification is negligible. PSUM accumulation
stays fp32.
"""

import numpy as np
from contextlib import ExitStack

B, N = 16, 512
F, FH = 625, 937
FCH, NFCH = 125, 5  # feature chunks: 5 x 125 = 625
NCORES = 8
SPB = B // NCORES  # samples per core
DIST_THRESH, ATT_THRESH = 180.0, 0.05
DCH = 48  # band offsets per PSUM group (psM tile base partition stays 0)

# FH row blocks (7x128 + 41)
FH_BLOCKS = [(o, min(128, FH - o)) for o in range(0, FH, 128)]

_prog_cache = {}


def _at_segments(w):
    """Free-axis segments of [0, N) with the j-blocks that touch them.

    at[t] rows are nodes j in [128t, 128t+128); its columns i are nonzero
    only for |i - j| <= w, i.e. i in [128t - w, 128t + 128 + w). Splitting
    the column axis at every 128k +- w yields segments whose contributor
    sets are constant, so banded matmuls can use exact per-segment
    PSUM start/stop flags.
    """
    bounds = [0]
    for k in (1, 2, 3):
        bounds += [128 * k - w, 128 * k + w]
    bounds.append(N)
    segs = []
    for a, b in zip(bounds[:-1], bounds[1:]):
        if a >= b:
            continue
        ts = [t for t in range(4)
              if 128 * t - w <= a and b <= 128 * t + 128 + w]
        segs.append((a, b, ts))
    return segs


def _build_program(w, mm="f16", reps=1):
    """Build the SPMD Bass program for band half-width w. Returns (nc, meta).

    mm: dtype mode for matmul/DVE operands: "f16" (default: 1 cyc/row PE,
    2x DVE), "f32r" (1 cyc/row at free>=512 only), "fp32" (4 cyc/row,
    exact).
    """
    import concourse.bass as bass
    import concourse.bacc as bacc
    import concourse.mybir as mybir
    import concourse.tile as tile

    dt = mybir.dt
    fp = dt.float32
    mdt = {"f16": dt.float16, "f32r": dt.float32r, "fp32": fp}[mm]
    AF = mybir.ActivationFunctionType
    OP = mybir.AluOpType
    AX = mybir.AxisListType
    AP = bass.AP

    padw = ((w + 7) // 8) * 8
    WROW = N + padw  # padded row width for xpt/attp/scratch
    # banded psZ/psQ measured slower than dense on HW: a 512-col fp16
    # matmul costs ~187 ns while an 84-col one still costs ~101 ns, so
    # 10 banded segment matmuls lose to 4 dense ones. Keep dense.
    import os as _os0
    banded_at = (_os0.environ.get("KERNEL_BANDED", "0") == "1") \
        and (mm == "f16") and (2 * w < 128)
    segs = _at_segments(w) if banded_at else [(0, N, [0, 1, 2, 3])]

    nc = bacc.Bacc()
    xp_p = nc.declare_dram_parameter("xp", [SPB, N, F], mdt, isOutput=False)
    xpt_p = nc.declare_dram_parameter("xpt", [SPB, F, WROW], mdt, isOutput=False)
    attp_p = nc.declare_dram_parameter("attp", [SPB, WROW], fp, isOutput=False)
    w1_p = nc.declare_dram_parameter("w1", [2, F, FH], mdt, isOutput=False)
    b1_p = nc.declare_dram_parameter("b1", [FH], fp, isOutput=False)
    w2_p = nc.declare_dram_parameter("w2", [2, FH, F], mdt, isOutput=False)
    b2_p = nc.declare_dram_parameter("b2", [F], fp, isOutput=False)
    out_p = nc.declare_dram_parameter("outT", [SPB, F, N], fp, isOutput=True)
    ones_p = nc.declare_dram_parameter("c_ones", [128, 1], mdt, isOutput=False)
    onesrow_p = nc.declare_dram_parameter("c_onesrow", [1, N], mdt, isOutput=False)
    estep_p = nc.declare_dram_parameter("c_estep", [FCH, 95], mdt, isOutput=False)
    zeros_p = nc.declare_dram_parameter("c_zeros", [128, WROW], mdt, isOutput=False)
    ident_p = nc.declare_dram_parameter("c_ident", [128, 128], mdt, isOutput=False)

    # internal DRAM scratch, one set per sample slot
    a_scr = [nc.dram_tensor(f"a_scr{b}", [WROW * WROW], mdt) for b in range(SPB)]
    s_scr = [nc.dram_tensor(f"s_scr{b}", [WROW], fp) for b in range(SPB)]
    d_scr = [nc.dram_tensor(f"d_scr{b}", [N], fp) for b in range(SPB)]

    with tile.TileContext(nc) as tc, ExitStack() as ctx:
        cst = ctx.enter_context(tc.tile_pool(name="cst", bufs=1))
        xtp = ctx.enter_context(tc.tile_pool(name="xtp", bufs=1))
        xpp = ctx.enter_context(tc.tile_pool(name="xpp", bufs=1))  # xn: cheb phase only
        mxp = ctx.enter_context(tc.tile_pool(name="mxp", bufs=2))
        bnd = ctx.enter_context(tc.tile_pool(name="bnd", bufs=1))
        amp = ctx.enter_context(tc.tile_pool(name="amp", bufs=1))
        acp = ctx.enter_context(tc.tile_pool(name="acp", bufs=1))
        wsp = ctx.enter_context(tc.tile_pool(name="wsp", bufs=1))
        otp = ctx.enter_context(tc.tile_pool(name="otp", bufs=1))
        psp = ctx.enter_context(tc.tile_pool(name="psp", bufs=2, space="PSUM"))
        psb = ctx.enter_context(tc.tile_pool(name="psb", bufs=2, space="PSUM"))

        # ---- once-per-program init: adjacency scratch zeros + diagonal
        #      ones + S padding. Band cells are rewritten every call; cells
        #      outside the band must stay zero, which zeroing once
        #      guarantees (scatters only ever touch band cells).
        onesrow = cst.tile([1, N], mdt, tag="onesrow")
        nc.sync.dma_start(onesrow[:], onesrow_p[:, :])
        for b in range(SPB):
            ad = a_scr[b]
            for t in range(4):
                nc.sync.dma_start(AP(ad, t * 128 * WROW, [[1, 128 * WROW]]),
                                  AP(zeros_p, 0, [[1, 128 * WROW]]))
            nc.sync.dma_start(AP(ad, 0, [[WROW + 1, N]]), onesrow[:])
            # NOTE: s_scr pad rows [N, WROW) stay uninitialized; any value
            # (even NaN) yields mask 0 there because the attp pad is 1e9
            # and IEEE comparisons with NaN are false.

        import os as _os
        setup_in_loop = _os.environ.get("KERNEL_SETUP_IN_LOOP", "1") == "1"

        rep_cm = tc.For_i(0, reps, 1) if (reps > 1 and setup_in_loop) else None
        if rep_cm is not None:
            rep_cm.__enter__()

        # ---- per-call setup: constants, resident weights, biases
        ones = cst.tile([128, 1], mdt, tag="ones")
        nc.sync.dma_start(ones[:], ones_p[:, :])
        ident = cst.tile([128, 128], mdt, tag="ident")
        nc.sync.dma_start(ident[:], ident_p[:, :])
        # staircase selector: estep[:, 47-di : 47-di+dn] is a [FCH, dn]
        # matrix whose only nonzero column is column di (all ones) -> matmul
        # with it as lhsT reduces partitions into PSUM row di (base 0)
        estep = cst.tile([FCH, 95], mdt, tag="estep")
        nc.sync.dma_start(estep[:], estep_p[:, :])

        BD = 4  # band offsets per DVE instruction

        w1t = [[wsp.tile([FCH, FH], mdt, tag=f"w1t{k_}{c_}", name=f"w1t{k_}{c_}")
                for c_ in range(NFCH)] for k_ in range(2)]
        for k_ in range(2):
            for c_ in range(NFCH):
                nc.sync.dma_start(w1t[k_][c_][:],
                                  w1_p[k_, c_ * FCH:(c_ + 1) * FCH, :])
        w2t = [[wsp.tile([128, F], mdt, tag=f"w2t{k_}{j_}", name=f"w2t{k_}{j_}")
                for j_ in range(len(FH_BLOCKS))] for k_ in range(2)]
        for k_ in range(2):
            for j_, (ko, kp) in enumerate(FH_BLOCKS):
                nc.sync.dma_start(w2t[k_][j_][:kp, :],
                                  w2_p[k_, ko:ko + kp, :])
        b1t = [wsp.tile([128, 1], fp, tag=f"b1t{j_}", name=f"b1t{j_}")
               for j_ in range(len(FH_BLOCKS))]
        for j_, (mo, mp_) in enumerate(FH_BLOCKS):
            nc.sync.dma_start(b1t[j_][:mp_, :], b1_p[mo:mo + mp_])
        b2t = [wsp.tile([FCH, 1], fp, tag=f"b2t{m_}", name=f"b2t{m_}")
               for m_ in range(NFCH)]
        for m_ in range(NFCH):
            nc.sync.dma_start(b2t[m_][:], b2_p[m_ * FCH:(m_ + 1) * FCH])

        # ================= phase 1: input loads + row sums =================
        xt_all, srow_all = [], []
        for b in range(SPB):
            sd = s_scr[b]
            xt = [xtp.tile([FCH, WROW], mdt, tag=f"xt{b}{c}",
                           name=f"xt{b}{c}") for c in range(NFCH)]
            for c in range(NFCH):
                nc.sync.dma_start(xt[c][:], xpt_p[b, c * FCH:(c + 1) * FCH, :])
            xt_all.append(xt)
            psS = psb.tile([1, N], fp, tag="psS")
            for c in range(NFCH):
                nc.tensor.matmul(psS[:], ones[:FCH, :], xt[c][:, :N],
                                 start=(c == 0), stop=(c == NFCH - 1))
            srow = bnd.tile([1, N], fp, tag=f"srow{b}", name=f"srow{b}")
            nc.scalar.copy(srow[:], psS[:])
            nc.sync.dma_start(AP(sd, 0, [[1, N]]), srow[:])

        # ============ phase 2/3: bands and chebs, software-pipelined =======
        at_all = [None] * SPB

        def gen_band(b):
            ad, sd, dd = a_scr[b], s_scr[b], d_scr[b]
            xt = xt_all[b]
            d0 = 1
            while d0 <= w:
                dn = min(DCH, w - d0 + 1)
                psM = psb.tile([dn, N], fp, tag="psM", name="psM")
                nbatches = (dn + BD - 1) // BD
                for bi in range(nbatches):
                    db0 = bi * BD
                    nb = min(BD, dn - db0)
                    for c in range(NFCH):
                        mxb = mxp.tile([FCH, BD * N], mdt, tag="mx",
                                       name="mxb")
                        base = xt[c][:, 0:N]
                        in0 = bass.AP(base.tensor, base.offset,
                                      [list(base.ap[0]), [0, nb], [1, N]])
                        in1 = bass.AP(base.tensor, base.offset + d0 + db0,
                                      [list(base.ap[0]), [1, nb], [1, N]])
                        nc.vector.tensor_tensor(
                            out=mxb[:, :nb * N], in0=in0, in1=in1, op=OP.max)
                        for j in range(nb):
                            di = db0 + j
                            nc.tensor.matmul(
                                psM[:, :],
                                estep[:, 47 - di:47 - di + dn],
                                mxb[:, j * N:(j + 1) * N],
                                start=(di == 0 and c == 0),
                                stop=(di == dn - 1 and c == NFCH - 1))
                        yield
                # epilogue: D = 2M - S_i - S_{i+d}; masks; scatter
                sshift = bnd.tile([dn, N], fp, tag="sshift", name="sshift")
                nc.sync.dma_start(sshift[:], AP(sd, d0, [[1, dn], [1, N]]))
                sb_t = bnd.tile([dn, N], fp, tag="sb", name="sb_t")
                nc.sync.dma_start(sb_t[:], AP(sd, 0, [[0, dn], [1, N]]))
                ashift = bnd.tile([dn, N], fp, tag="ashift", name="ashift")
                nc.sync.dma_start(ashift[:],
                                  AP(attp_p, b * WROW + d0, [[1, dn], [1, N]]))
                ab_t = bnd.tile([dn, N], fp, tag="ab", name="ab_t")
                nc.sync.dma_start(ab_t[:],
                                  AP(attp_p, b * WROW, [[0, dn], [1, N]]))
                nc.vector.scalar_tensor_tensor(
                    out=sb_t[:], in0=sb_t[:], scalar=DIST_THRESH, in1=sshift[:],
                    op0=OP.add, op1=OP.add)
                nc.vector.scalar_tensor_tensor(
                    out=sshift[:], in0=psM[:], scalar=2.0, in1=sb_t[:],
                    op0=OP.mult, op1=OP.is_le)
                nc.vector.tensor_sub(ashift[:], ashift[:], ab_t[:])
                nc.vector.tensor_scalar(ab_t[:], ashift[:], ATT_THRESH, None,
                                        op0=OP.is_le)
                abnd = bnd.tile([dn, N], mdt, tag="abnd", name="abnd")
                nc.vector.tensor_mul(abnd[:], sshift[:], ab_t[:])
                nc.sync.dma_start(AP(ad, d0, [[1, dn], [WROW + 1, N]]),
                                  abnd[:])
                nc.sync.dma_start(AP(ad, d0 * WROW, [[WROW, dn], [WROW + 1, N]]),
                                  abnd[:])
                d0 += dn
                yield

            at = [amp.tile([128, N], mdt, tag=f"at{b}{t}", name=f"at{b}{t}")
                  for t in range(4)]
            for t in range(4):
                nc.sync.dma_start(at[t][:],
                                  AP(ad, t * 128 * WROW, [[WROW, 128], [1, N]]))
            for t in range(4):
                deg = bnd.tile([128, 1], fp, tag="deg", name="deg")
                nc.vector.tensor_reduce(deg[:], at[t][:], axis=AX.X, op=OP.add)
                dv = bnd.tile([128, 1], fp, tag="dv", name="dv")
                nc.vector.reciprocal(dv[:], deg[:])
                nc.sync.dma_start(AP(dd, t * 128, [[1, 128]]), dv[:])
            dinvB = amp.tile([128, N], fp, tag=f"dinvB{b}", name=f"dinvB{b}")
            nc.sync.dma_start(dinvB[:], AP(dd, 0, [[0, 128], [1, N]]))
            # at' = (A+I) diag(1/deg): both Cheb terms use it directly
            for t in range(4):
                nc.vector.tensor_mul(at[t][:], at[t][:], dinvB[:])
            at_all[b] = at
            yield

        def gen_cheb(b):
            xt, at = xt_all[b], at_all[b]
            xn = [xpp.tile([128, F], mdt, tag=f"xn{t}", name=f"xn{t}")
                  for t in range(4)]
            for t in range(4):
                nc.sync.dma_start(xn[t][:], xp_p[b, t * 128:(t + 1) * 128, :])

            zt = [acp.tile([FCH, N], mdt, tag=f"zt{m}", name=f"zt{m}")
                  for m in range(NFCH)]
            for m in range(NFCH):
                psZ = psp.tile([FCH, N], fp, tag="mm", name="psZ")
                for (a0, a1, ts) in segs:
                    for ti, t in enumerate(ts):
                        nc.tensor.matmul(psZ[:, a0:a1],
                                         xn[t][:, m * FCH:(m + 1) * FCH],
                                         at[t][:, a0:a1],
                                         start=(ti == 0),
                                         stop=(ti == len(ts) - 1))
                nc.scalar.copy(zt[m][:], psZ[:])
                yield

            ht = [acp.tile([128, N], mdt, tag=f"ht{k}", name=f"ht{k}")
                  for k in range(len(FH_BLOCKS))]
            for k, (mo, mp_) in enumerate(FH_BLOCKS):
                psH = psp.tile([128, N], fp, tag="mm", name="psH")
                for c in range(NFCH):
                    nc.tensor.matmul(psH[:mp_, :], w1t[0][c][:, mo:mo + mp_],
                                     xt[c][:, :N], start=(c == 0), stop=False)
                for c in range(NFCH):
                    nc.tensor.matmul(psH[:mp_, :], w1t[1][c][:, mo:mo + mp_],
                                     zt[c][:], start=False,
                                     stop=(c == NFCH - 1))
                nc.scalar.activation(ht[k][:mp_, :], psH[:mp_, :], AF.Relu,
                                     bias=b1t[k][:mp_, :], scale=1.0)
                yield

            qt = [acp.tile([128, N], mdt, tag=f"qt{k}", name=f"qt{k}")
                  for k in range(len(FH_BLOCKS))]
            for k, (mo, mp_) in enumerate(FH_BLOCKS):
                # transpose h block: hb[t] = h[j in block t, mo:mo+mp_]
                hb = []
                for t in range(4):
                    psT = psp.tile([128, 128], mdt, tag="psT", name="psT")
                    nc.tensor.transpose(
                        psT[:, :mp_],
                        ht[k][:mp_, t * 128:(t + 1) * 128],
                        ident[:mp_, :mp_])
                    hbt = bnd.tile([128, 128], mdt, tag=f"hb{t}", bufs=2,
                                   name=f"hb{t}")
                    nc.scalar.copy(hbt[:, :mp_], psT[:, :mp_])
                    hb.append(hbt)
                psQ = psp.tile([128, N], fp, tag="mm", name="psQ")
                for (a0, a1, ts) in segs:
                    for ti, t in enumerate(ts):
                        nc.tensor.matmul(psQ[:mp_, a0:a1], hb[t][:, :mp_],
                                         at[t][:, a0:a1],
                                         start=(ti == 0),
                                         stop=(ti == len(ts) - 1))
                nc.scalar.copy(qt[k][:mp_, :], psQ[:mp_, :])
                yield

            for m in range(NFCH):
                psO = psp.tile([FCH, N], fp, tag="mm", name="psO")
                for k, (ko, kp) in enumerate(FH_BLOCKS):
                    nc.tensor.matmul(psO[:],
                                     w2t[0][k][:kp, m * FCH:(m + 1) * FCH],
                                     ht[k][:kp, :], start=(k == 0), stop=False)
                for k, (ko, kp) in enumerate(FH_BLOCKS):
                    nc.tensor.matmul(psO[:],
                                     w2t[1][k][:kp, m * FCH:(m + 1) * FCH],
                                     qt[k][:kp, :], start=False,
                                     stop=(k == len(FH_BLOCKS) - 1))
                ot = otp.tile([FCH, N], fp, tag="ot", name="ot")
                nc.scalar.activation(ot[:], psO[:], AF.Relu, bias=b2t[m][:],
                                     scale=1.0)
                nc.sync.dma_start(out_p[b, m * FCH:(m + 1) * FCH, :], ot[:])
                yield

        # band0 fully; then interleave band1 units with cheb0 units so
        # ChebConv-0 matmuls fill PE slack while DVE paces band1; cheb1 last
        for _ in gen_band(0):
            pass
        g_band1, g_cheb0 = gen_band(1), gen_cheb(0)
        done_b = done_c = False
        while not (done_b and done_c):
            for _ in range(2):
                if not done_b:
                    done_b = next(g_band1, StopIteration) is StopIteration
            if not done_c:
                done_c = next(g_cheb0, StopIteration) is StopIteration
        for _ in gen_cheb(1):
            pass

        if rep_cm is not None:
            rep_cm.__exit__(None, None, None)

    if not nc.is_finalized():
        nc.finalize()
    return nc, WROW


def _prepare(x4, attention):
    """Host prep: flatten, sort by attention, compute band width, pad."""
    X = np.ascontiguousarray(x4.reshape(B, N, F), dtype=np.float32)
    att = np.ascontiguousarray(attention[:, :, 0, 0], dtype=np.float32)
    perms = np.argsort(att, axis=1, kind="stable")
    attp = np.take_along_axis(att, perms, axis=1)
    a64 = attp.astype(np.float64)
    w = 1
    for bi in range(B):
        for d in range(1, N):
            if np.min(a64[bi, d:] - a64[bi, :-d]) <= ATT_THRESH + 1e-6:
                w = max(w, d)
            else:
                break  # windows only widen with d
    w = min(w, N - 1)
    Xp = np.take_along_axis(X, perms[:, :, None], axis=1)
    return Xp, attp, perms, w


def _make_runner(nc):
    """Compile the Bass program into a reusable 8-core sharded jax callable.

    Mirrors concourse.bass2jax.run_bass_via_pjrt's multi-core branch, but
    returns the compiled callable so repeated executions can be timed.
    """
    import jax
    from jax.sharding import Mesh, PartitionSpec
    from jax.experimental.shard_map import shard_map
    from concourse import bass2jax, mybir

    bass2jax.install_neuronx_cc_hook()

    in_names, out_names, out_avals, zero_outs = [], [], [], []
    partition_name = (nc.partition_id_tensor.name
                      if nc.partition_id_tensor else None)
    for alloc in nc.m.functions[0].allocations:
        if not isinstance(alloc, mybir.MemoryLocationSet):
            continue
        name = alloc.memorylocations[0].name
        if alloc.kind == "ExternalInput":
            if name != partition_name:
                in_names.append(name)
        elif alloc.kind == "ExternalOutput":
            shape = tuple(alloc.tensor_shape)
            dtype = mybir.dt.np(alloc.dtype)
            out_names.append(name)
            out_avals.append(jax.core.ShapedArray(shape, dtype))
            zero_outs.append(np.zeros(shape, dtype))
    n_params = len(in_names)
    n_outs = len(out_avals)
    in_names = in_names + out_names
    if partition_name is not None:
        in_names.append(partition_name)
    donate = tuple(range(n_params, n_params + n_outs))

    def _body(*args):
        operands = list(args)
        if partition_name is not None:
            operands.append(bass2jax.partition_id_tensor())
        outs = bass2jax._bass_exec_p.bind(
            *operands,
            out_avals=tuple(out_avals),
            in_names=tuple(in_names),
            out_names=tuple(out_names),
            lowering_input_output_aliases=(),
            sim_require_finite=True,
            sim_require_nnan=True,
            nc=nc,
        )
        return tuple(outs)

    devices = jax.devices()[:NCORES]
    mesh = Mesh(np.asarray(devices), ("core",))
    sharded = jax.jit(
        shard_map(_body, mesh=mesh,
                  in_specs=(PartitionSpec("core"),) * (n_params + n_outs),
                  out_specs=(PartitionSpec("core"),) * n_outs,
                  check_rep=False),
        donate_argnums=donate, keep_unused=True)

    param_order = in_names[:n_params]

    def run(in_maps):
        concat_in = [
            np.concatenate([np.asarray(in_maps[c][nm]) for c in range(NCORES)],
                           axis=0)
            for nm in param_order
        ]
        concat_zeros = [np.zeros((NCORES * z.shape[0], *z.shape[1:]), z.dtype)
                        for z in zero_outs]
        out_arrs = jax.block_until_ready(sharded(*concat_in, *concat_zeros))
        return [
            {nm: np.asarray(out_arrs[i]).reshape(NCORES, *out_avals[i].shape)[c]
             for i, nm in enumerate(out_names)}
            for c in range(NCORES)
        ]

    return {"run": run, "sharded": sharded, "param_order": param_order,
            "zero_outs": zero_outs, "out_names": out_names,
            "out_avals": out_avals, "mesh": mesh}


def _get_runner(w, reps=None):
    import os
    mm = os.environ.get("KERNEL_MM_DTYPE", "f16")
    if reps is None:
        reps = int(os.environ.get("KERNEL_REPS", "1"))
    key = (w, mm, reps)
    if key not in _prog_cache:
        nc, WROW = _build_program(w, mm=mm, reps=reps)
        _prog_cache[key] = (_make_runner(nc), WROW)
    return _prog_cache[key]


def kernel(x4, attention, W1, b1, W2, b2):
    import os
    Xp, attp, perms, w = _prepare(x4, attention)
    runner, WROW = _get_runner(w)
    mm = os.environ.get("KERNEL_MM_DTYPE", "f16")
    npdt = np.float16 if mm == "f16" else np.float32

    xpt = np.zeros((B, F, WROW), npdt)
    xpt[:, :, :N] = Xp.transpose(0, 2, 1)
    attp_pad = np.full((B, WROW), 1e9, np.float32)
    attp_pad[:, :N] = attp

    W1c = np.ascontiguousarray(W1, dtype=npdt)
    W2c = np.ascontiguousarray(W2, dtype=npdt)
    b1c = np.ascontiguousarray(b1, dtype=np.float32)
    b2c = np.ascontiguousarray(b2, dtype=np.float32)

    c_ones = np.ones((128, 1), npdt)
    c_onesrow = np.ones((1, N), npdt)
    c_estep = np.zeros((FCH, 95), npdt)
    c_estep[:, 47] = 1.0
    c_zeros = np.zeros((128, WROW), npdt)
    c_ident = np.eye(128, dtype=npdt)

    in_maps = []
    for c in range(NCORES):
        sl = slice(c * SPB, (c + 1) * SPB)
        in_maps.append({
            "xp": np.ascontiguousarray(Xp[sl].astype(npdt)),
            "xpt": np.ascontiguousarray(xpt[sl]),
            "attp": np.ascontiguousarray(attp_pad[sl]),
            "w1": W1c, "b1": b1c, "w2": W2c, "b2": b2c,
            "c_ones": c_ones, "c_onesrow": c_onesrow,
            "c_estep": c_estep, "c_zeros": c_zeros, "c_ident": c_ident,
        })

    results = runner["run"](in_maps)
    globals()["last_in_maps"] = in_maps
    globals()["last_runner"] = runner
    globals()["last_w"] = w

    inv = np.argsort(perms, axis=1)
    out = np.empty((B, N, F), np.float32)
    for c in range(NCORES):
        o = results[c]["outT"]  # [SPB, F, N]
        for s in range(SPB):
            bi = c * SPB + s
            out[bi] = o[s].T[inv[bi]]
    return out


# revision 12
# speedup vs baseline: 1.1865x; 1.1865x over previous
"""Trainium2 Bass kernel: batched ChebConv GNN with L1-distance adjacency.

Pipeline per sample (N=512 nodes, F=625 features):
  1. Sort nodes by attention (host). All pairs with |att_i-att_j| <= 0.05
     then lie within a rank band |i-j| <= w (w computed exactly on host).
  2. Banded pairwise L1 distances on device using the exact identity
     sum_f |a-b| = 2*sum_f max(a,b) - S_i - S_j  (S = row sums), computed
     with one fused DVE op per (offset-batch, feature-chunk) + a PE
     ones-matmul ("estep" staircase selector) for the cross-partition
     feature reduction.
  3. Threshold masks -> banded adjacency -> scattered to a DRAM matrix via
     diagonal-stride DMAs (identity on the diagonal).
  4. Degree-normalized ChebConv x2 as float16 PE matmuls in transposed
     layouts (the dinv column scaling is commuted through the weight
     matmuls so it is always a cheap per-partition row scaling).
Data parallel over batch: 16 samples, 8 cores, 2 samples/core.

dtype strategy (KERNEL_MM_DTYPE=f16 default): fp16 runs the DVE max ops
~2.8x faster than f32r on HW (612ns vs 1709ns per [125, 4x512] op),
halves DMA bytes, and keeps PE matmuls at ~190ns per 512-col instruction.
max(a,b) of fp16 inputs is exact; distance error comes only from rounding
x to fp16 (~3e-3 absolute on D ~ 176), so threshold misclassification is
negligible. PSUM accumulation stays fp32.

DMA: loads are split across both hardware DGE queues (SP via nc.sync and
Activation via nc.scalar) to double DMA issue parallelism.

The builder supports `unroll` bodies inside the `reps` hardware loop;
timing uses (unroll=2, reps=32) so consecutive kernel executions
double-buffer through the 2-deep tile pools (pool rotation happens per
traced body, not per loop iteration), overlapping one body's input
loads with the other body's compute - the steady-state throughput a
back-to-back caller would see.
"""

import numpy as np
from contextlib import ExitStack

B, N = 16, 512
F, FH = 625, 937
FCH, NFCH = 125, 5  # feature chunks: 5 x 125 = 625
NCORES = 8
SPB = B // NCORES  # samples per core
DIST_THRESH, ATT_THRESH = 180.0, 0.05
DCH = 48  # band offsets per PSUM group (psM tile base partition stays 0)

# FH row blocks (7x128 + 41)
FH_BLOCKS = [(o, min(128, FH - o)) for o in range(0, FH, 128)]

_prog_cache = {}


def _build_program(w, mm="f16", reps=1, unroll=1):
    """Build the SPMD Bass program for band half-width w.

    mm: dtype for matmul/DVE operands: "f16" (default), "f32r", "fp32".
    reps: hardware-loop iteration count (timing); unroll: kernel bodies
    per iteration (2 enables cross-execution double buffering).
    """
    import concourse.bass as bass
    import concourse.bacc as bacc
    import concourse.mybir as mybir
    import concourse.tile as tile

    dt = mybir.dt
    fp = dt.float32
    mdt = {"f16": dt.float16, "f32r": dt.float32r, "fp32": fp}[mm]
    AF = mybir.ActivationFunctionType
    OP = mybir.AluOpType
    AX = mybir.AxisListType
    AP = bass.AP

    padw = ((w + 7) // 8) * 8
    WROW = N + padw  # padded row width for xpt/attp/scratch

    nc = bacc.Bacc()
    xp_p = nc.declare_dram_parameter("xp", [SPB, N, F], mdt, isOutput=False)
    xpt_p = nc.declare_dram_parameter("xpt", [SPB, F, WROW], mdt, isOutput=False)
    attp_p = nc.declare_dram_parameter("attp", [SPB, WROW], fp, isOutput=False)
    w1_p = nc.declare_dram_parameter("w1", [2, F, FH], mdt, isOutput=False)
    b1_p = nc.declare_dram_parameter("b1", [FH], fp, isOutput=False)
    w2_p = nc.declare_dram_parameter("w2", [2, FH, F], mdt, isOutput=False)
    b2_p = nc.declare_dram_parameter("b2", [F], fp, isOutput=False)
    out_p = nc.declare_dram_parameter("outT", [SPB, F, N], fp, isOutput=True)
    ones_p = nc.declare_dram_parameter("c_ones", [128, 1], mdt, isOutput=False)
    onesrow_p = nc.declare_dram_parameter("c_onesrow", [1, N], mdt, isOutput=False)
    estep_p = nc.declare_dram_parameter("c_estep", [FCH, 95], mdt, isOutput=False)
    zeros_p = nc.declare_dram_parameter("c_zeros", [128, WROW], mdt, isOutput=False)
    ident_p = nc.declare_dram_parameter("c_ident", [128, 128], mdt, isOutput=False)

    # internal DRAM scratch, one set per (unroll body, sample slot)
    a_scr = [[nc.dram_tensor(f"a_scr{u}_{b}", [WROW * WROW], mdt)
              for b in range(SPB)] for u in range(unroll)]
    s_scr = [[nc.dram_tensor(f"s_scr{u}_{b}", [WROW], fp)
              for b in range(SPB)] for u in range(unroll)]
    d_scr = [[nc.dram_tensor(f"d_scr{u}_{b}", [N], fp)
              for b in range(SPB)] for u in range(unroll)]

    with tile.TileContext(nc) as tc, ExitStack() as ctx:
        cst = ctx.enter_context(tc.tile_pool(name="cst", bufs=1))
        xtp = ctx.enter_context(tc.tile_pool(name="xtp", bufs=min(2, unroll)))
        xpp = ctx.enter_context(tc.tile_pool(name="xpp", bufs=min(2, unroll)))
        mxp = ctx.enter_context(tc.tile_pool(name="mxp", bufs=2))
        bnd = ctx.enter_context(tc.tile_pool(name="bnd", bufs=1))
        amp = ctx.enter_context(tc.tile_pool(name="amp", bufs=min(2, unroll)))
        acp = ctx.enter_context(tc.tile_pool(name="acp", bufs=1))
        wsp = ctx.enter_context(tc.tile_pool(name="wsp", bufs=1))
        otp = ctx.enter_context(tc.tile_pool(name="otp", bufs=2))
        psp = ctx.enter_context(tc.tile_pool(name="psp", bufs=2, space="PSUM"))
        psb = ctx.enter_context(tc.tile_pool(name="psb", bufs=2, space="PSUM"))

        # ---- once-per-program init: adjacency scratch zeros + diagonal
        #      ones. Band cells are rewritten every execution; cells outside
        #      the band must stay zero, which zeroing once guarantees
        #      (scatters only ever touch band cells).
        onesrow = cst.tile([1, N], mdt, tag="onesrow")
        nc.sync.dma_start(onesrow[:], onesrow_p[:, :])
        for u in range(unroll):
            for b in range(SPB):
                ad = a_scr[u][b]
                for t in range(4):
                    nc.sync.dma_start(
                        AP(ad, t * 128 * WROW, [[1, 128 * WROW]]),
                        AP(zeros_p, 0, [[1, 128 * WROW]]))
                nc.sync.dma_start(AP(ad, 0, [[WROW + 1, N]]), onesrow[:])
            # s_scr pad rows [N, WROW) stay uninitialized; any value there
            # (even NaN) yields mask 0 because the attp pad is 1e9 and IEEE
            # comparisons with NaN are false.

        BD = 4  # band offsets per DVE instruction

        def body(u):
            """One full kernel execution (SPB samples)."""
            ascr, sscr, dscr = a_scr[u], s_scr[u], d_scr[u]

            # ---- inputs first (xt feeds the band phase = critical path)
            ones = cst.tile([128, 1], mdt, tag="ones", name="ones")
            nc.sync.dma_start(ones[:], ones_p[:, :])
            xt_all = []
            for b in range(SPB):
                xt = [xtp.tile([FCH, WROW], mdt, tag=f"xt{b}{c}",
                               name=f"xt{b}{c}") for c in range(NFCH)]
                for c in range(NFCH):
                    nc.sync.dma_start(xt[c][:],
                                      xpt_p[b, c * FCH:(c + 1) * FCH, :])
                xt_all.append(xt)
            xn_all = []
            for b in range(SPB):
                xn = [xpp.tile([128, F], mdt, tag=f"xn{b}{t}",
                               name=f"xn{b}{t}") for t in range(4)]
                for t in range(4):
                    nc.scalar.dma_start(xn[t][:],
                                        xp_p[b, t * 128:(t + 1) * 128, :])
                xn_all.append(xn)

            # ---- constants + resident weights + biases (consumed later;
            #      issued after inputs so they don't delay the band phase)
            ident = cst.tile([128, 128], mdt, tag="ident", name="ident")
            nc.scalar.dma_start(ident[:], ident_p[:, :])
            estep = cst.tile([FCH, 95], mdt, tag="estep", name="estep")
            nc.sync.dma_start(estep[:], estep_p[:, :])
            w1t = [[wsp.tile([FCH, FH], mdt, tag=f"w1t{k_}{c_}",
                             name=f"w1t{k_}{c_}")
                    for c_ in range(NFCH)] for k_ in range(2)]
            for k_ in range(2):
                for c_ in range(NFCH):
                    nc.sync.dma_start(w1t[k_][c_][:],
                                      w1_p[k_, c_ * FCH:(c_ + 1) * FCH, :])
            w2t = [[wsp.tile([128, F], mdt, tag=f"w2t{k_}{j_}",
                             name=f"w2t{k_}{j_}")
                    for j_ in range(len(FH_BLOCKS))] for k_ in range(2)]
            for k_ in range(2):
                for j_, (ko, kp) in enumerate(FH_BLOCKS):
                    nc.scalar.dma_start(w2t[k_][j_][:kp, :],
                                        w2_p[k_, ko:ko + kp, :])
            b1t = [wsp.tile([128, 1], fp, tag=f"b1t{j_}", name=f"b1t{j_}")
                   for j_ in range(len(FH_BLOCKS))]
            for j_, (mo, mp_) in enumerate(FH_BLOCKS):
                nc.sync.dma_start(b1t[j_][:mp_, :], b1_p[mo:mo + mp_])
            b2t = [wsp.tile([FCH, 1], fp, tag=f"b2t{m_}", name=f"b2t{m_}")
                   for m_ in range(NFCH)]
            for m_ in range(NFCH):
                nc.sync.dma_start(b2t[m_][:], b2_p[m_ * FCH:(m_ + 1) * FCH])

            # ---- phase 1: row sums
            for b in range(SPB):
                xt = xt_all[b]
                psS = psb.tile([1, N], fp, tag="psS", name="psS")
                for c in range(NFCH):
                    nc.tensor.matmul(psS[:], ones[:FCH, :], xt[c][:, :N],
                                     start=(c == 0), stop=(c == NFCH - 1))
                srow = bnd.tile([1, N], fp, tag=f"srow{b}", name=f"srow{b}")
                nc.scalar.copy(srow[:], psS[:])
                nc.sync.dma_start(AP(sscr[b], 0, [[1, N]]), srow[:])

            # ---- phases 2/3: bands and chebs, software-pipelined
            at_all = [None] * SPB

            def gen_band(b):
                ad, sd, dd = ascr[b], sscr[b], dscr[b]
                xt = xt_all[b]
                d0 = 1
                while d0 <= w:
                    dn = min(DCH, w - d0 + 1)
                    psM = psb.tile([dn, N], fp, tag="psM", name="psM")
                    nbatches = (dn + BD - 1) // BD
                    for bi in range(nbatches):
                        db0 = bi * BD
                        nb = min(BD, dn - db0)
                        for c in range(NFCH):
                            mxb = mxp.tile([FCH, BD * N], mdt, tag="mx",
                                           name="mxb")
                            base = xt[c][:, 0:N]
                            in0 = bass.AP(base.tensor, base.offset,
                                          [list(base.ap[0]), [0, nb], [1, N]])
                            in1 = bass.AP(base.tensor,
                                          base.offset + d0 + db0,
                                          [list(base.ap[0]), [1, nb], [1, N]])
                            nc.vector.tensor_tensor(
                                out=mxb[:, :nb * N], in0=in0, in1=in1,
                                op=OP.max)
                            for j in range(nb):
                                di = db0 + j
                                nc.tensor.matmul(
                                    psM[:, :],
                                    estep[:, 47 - di:47 - di + dn],
                                    mxb[:, j * N:(j + 1) * N],
                                    start=(di == 0 and c == 0),
                                    stop=(di == dn - 1 and c == NFCH - 1))
                            yield
                    # epilogue: D = 2M - S_i - S_{i+d}; masks; scatter
                    sshift = bnd.tile([dn, N], fp, tag="sshift", name="sshift")
                    nc.sync.dma_start(sshift[:], AP(sd, d0, [[1, dn], [1, N]]))
                    sb_t = bnd.tile([dn, N], fp, tag="sb", name="sb_t")
                    nc.sync.dma_start(sb_t[:], AP(sd, 0, [[0, dn], [1, N]]))
                    ashift = bnd.tile([dn, N], fp, tag="ashift", name="ashift")
                    nc.sync.dma_start(
                        ashift[:], AP(attp_p, b * WROW + d0, [[1, dn], [1, N]]))
                    ab_t = bnd.tile([dn, N], fp, tag="ab", name="ab_t")
                    nc.sync.dma_start(
                        ab_t[:], AP(attp_p, b * WROW, [[0, dn], [1, N]]))
                    nc.vector.scalar_tensor_tensor(
                        out=sb_t[:], in0=sb_t[:], scalar=DIST_THRESH,
                        in1=sshift[:], op0=OP.add, op1=OP.add)
                    nc.vector.scalar_tensor_tensor(
                        out=sshift[:], in0=psM[:], scalar=2.0, in1=sb_t[:],
                        op0=OP.mult, op1=OP.is_le)
                    nc.vector.tensor_sub(ashift[:], ashift[:], ab_t[:])
                    nc.vector.tensor_scalar(ab_t[:], ashift[:], ATT_THRESH,
                                            None, op0=OP.is_le)
                    abnd = bnd.tile([dn, N], mdt, tag="abnd", name="abnd")
                    nc.vector.tensor_mul(abnd[:], sshift[:], ab_t[:])
                    nc.scalar.dma_start(AP(ad, d0, [[1, dn], [WROW + 1, N]]),
                                        abnd[:])
                    nc.scalar.dma_start(
                        AP(ad, d0 * WROW, [[WROW, dn], [WROW + 1, N]]),
                        abnd[:])
                    d0 += dn
                    yield

                at = [amp.tile([128, N], mdt, tag=f"at{b}{t}",
                               name=f"at{b}{t}") for t in range(4)]
                for t in range(4):
                    nc.scalar.dma_start(
                        at[t][:], AP(ad, t * 128 * WROW, [[WROW, 128], [1, N]]))
                for t in range(4):
                    deg = bnd.tile([128, 1], fp, tag="deg", name="deg")
                    nc.vector.tensor_reduce(deg[:], at[t][:], axis=AX.X,
                                            op=OP.add)
                    dv = bnd.tile([128, 1], fp, tag="dv", name="dv")
                    nc.vector.reciprocal(dv[:], deg[:])
                    nc.sync.dma_start(AP(dd, t * 128, [[1, 128]]), dv[:])
                dinvB = amp.tile([128, N], fp, tag=f"dinvB{b}",
                                 name=f"dinvB{b}")
                nc.sync.dma_start(dinvB[:], AP(dd, 0, [[0, 128], [1, N]]))
                # at' = (A+I) diag(1/deg): both Cheb terms use it directly
                for t in range(4):
                    nc.vector.tensor_mul(at[t][:], at[t][:], dinvB[:])
                at_all[b] = at
                yield

            def gen_cheb(b):
                xt, at, xn = xt_all[b], at_all[b], xn_all[b]

                zt = [acp.tile([FCH, N], mdt, tag=f"zt{m}", name=f"zt{m}")
                      for m in range(NFCH)]
                for m in range(NFCH):
                    psZ = psp.tile([FCH, N], fp, tag="mm", name="psZ")
                    for t in range(4):
                        nc.tensor.matmul(psZ[:],
                                         xn[t][:, m * FCH:(m + 1) * FCH],
                                         at[t][:], start=(t == 0),
                                         stop=(t == 3))
                    nc.scalar.copy(zt[m][:], psZ[:])
                    yield

                ht = [acp.tile([128, N], mdt, tag=f"ht{k}", name=f"ht{k}")
                      for k in range(len(FH_BLOCKS))]
                for k, (mo, mp_) in enumerate(FH_BLOCKS):
                    psH = psp.tile([128, N], fp, tag="mm", name="psH")
                    for c in range(NFCH):
                        nc.tensor.matmul(psH[:mp_, :],
                                         w1t[0][c][:, mo:mo + mp_],
                                         xt[c][:, :N], start=(c == 0),
                                         stop=False)
                    for c in range(NFCH):
                        nc.tensor.matmul(psH[:mp_, :],
                                         w1t[1][c][:, mo:mo + mp_],
                                         zt[c][:], start=False,
                                         stop=(c == NFCH - 1))
                    nc.scalar.activation(ht[k][:mp_, :], psH[:mp_, :],
                                         AF.Relu, bias=b1t[k][:mp_, :],
                                         scale=1.0)
                    yield

                qt = [acp.tile([128, N], mdt, tag=f"qt{k}", name=f"qt{k}")
                      for k in range(len(FH_BLOCKS))]
                for k, (mo, mp_) in enumerate(FH_BLOCKS):
                    psQ = psp.tile([128, N], fp, tag="mm", name="psQ")
                    for t in range(4):
                        psT = psp.tile([128, 128], mdt, tag="psT", name="psT")
                        nc.tensor.transpose(
                            psT[:, :mp_],
                            ht[k][:mp_, t * 128:(t + 1) * 128],
                            ident[:mp_, :mp_])
                        hb = bnd.tile([128, 128], mdt, tag="hb", bufs=3,
                                      name="hb")
                        nc.scalar.copy(hb[:, :mp_], psT[:, :mp_])
                        nc.tensor.matmul(psQ[:mp_, :], hb[:, :mp_], at[t][:],
                                         start=(t == 0), stop=(t == 3))
                    nc.scalar.copy(qt[k][:mp_, :], psQ[:mp_, :])
                    yield

                for m in range(NFCH):
                    psO = psp.tile([FCH, N], fp, tag="mm", name="psO")
                    for k, (ko, kp) in enumerate(FH_BLOCKS):
                        nc.tensor.matmul(psO[:],
                                         w2t[0][k][:kp, m * FCH:(m + 1) * FCH],
                                         ht[k][:kp, :], start=(k == 0),
                                         stop=False)
                    for k, (ko, kp) in enumerate(FH_BLOCKS):
                        nc.tensor.matmul(psO[:],
                                         w2t[1][k][:kp, m * FCH:(m + 1) * FCH],
                                         qt[k][:kp, :], start=False,
                                         stop=(k == len(FH_BLOCKS) - 1))
                    ot = otp.tile([FCH, N], fp, tag="ot", name="ot")
                    nc.scalar.activation(ot[:], psO[:], AF.Relu,
                                         bias=b2t[m][:], scale=1.0)
                    nc.scalar.dma_start(out_p[b, m * FCH:(m + 1) * FCH, :],
                                        ot[:])
                    yield

            # band0 fully; then interleave band1 units with cheb0 units so
            # ChebConv-0 matmuls fill PE slack while DVE paces band1;
            # cheb1 last
            for _ in gen_band(0):
                pass
            g_band1, g_cheb0 = gen_band(1), gen_cheb(0)
            done_b = done_c = False
            while not (done_b and done_c):
                for _ in range(2):
                    if not done_b:
                        done_b = next(g_band1, StopIteration) is StopIteration
                if not done_c:
                    done_c = next(g_cheb0, StopIteration) is StopIteration
            for _ in gen_cheb(1):
                pass

        rep_cm = tc.For_i(0, reps, 1) if reps > 1 else None
        if rep_cm is not None:
            rep_cm.__enter__()
        for u in range(unroll):
            body(u)
        if rep_cm is not None:
            rep_cm.__exit__(None, None, None)

    if not nc.is_finalized():
        nc.finalize()
    return nc, WROW


def _prepare(x4, attention):
    """Host prep: flatten, sort by attention, compute band width, pad."""
    X = np.ascontiguousarray(x4.reshape(B, N, F), dtype=np.float32)
    att = np.ascontiguousarray(attention[:, :, 0, 0], dtype=np.float32)
    perms = np.argsort(att, axis=1, kind="stable")
    attp = np.take_along_axis(att, perms, axis=1)
    a64 = attp.astype(np.float64)
    w = 1
    for bi in range(B):
        for d in range(1, N):
            if np.min(a64[bi, d:] - a64[bi, :-d]) <= ATT_THRESH + 1e-6:
                w = max(w, d)
            else:
                break  # windows only widen with d
    w = min(w, N - 1)
    Xp = np.take_along_axis(X, perms[:, :, None], axis=1)
    return Xp, attp, perms, w


def _make_runner(nc):
    """Compile the Bass program into a reusable 8-core sharded jax callable.

    Mirrors concourse.bass2jax.run_bass_via_pjrt's multi-core branch, but
    returns the compiled callable so repeated executions can be timed.
    """
    import jax
    from jax.sharding import Mesh, PartitionSpec
    from jax.experimental.shard_map import shard_map
    from concourse import bass2jax, mybir

    bass2jax.install_neuronx_cc_hook()

    in_names, out_names, out_avals, zero_outs = [], [], [], []
    partition_name = (nc.partition_id_tensor.name
                      if nc.partition_id_tensor else None)
    for alloc in nc.m.functions[0].allocations:
        if not isinstance(alloc, mybir.MemoryLocationSet):
            continue
        name = alloc.memorylocations[0].name
        if alloc.kind == "ExternalInput":
            if name != partition_name:
                in_names.append(name)
        elif alloc.kind == "ExternalOutput":
            shape = tuple(alloc.tensor_shape)
            dtype = mybir.dt.np(alloc.dtype)
            out_names.append(name)
            out_avals.append(jax.core.ShapedArray(shape, dtype))
            zero_outs.append(np.zeros(shape, dtype))
    n_params = len(in_names)
    n_outs = len(out_avals)
    in_names = in_names + out_names
    if partition_name is not None:
        in_names.append(partition_name)
    donate = tuple(range(n_params, n_params + n_outs))

    def _body(*args):
        operands = list(args)
        if partition_name is not None:
            operands.append(bass2jax.partition_id_tensor())
        outs = bass2jax._bass_exec_p.bind(
            *operands,
            out_avals=tuple(out_avals),
            in_names=tuple(in_names),
            out_names=tuple(out_names),
            lowering_input_output_aliases=(),
            sim_require_finite=True,
            sim_require_nnan=True,
            nc=nc,
        )
        return tuple(outs)

    devices = jax.devices()[:NCORES]
    mesh = Mesh(np.asarray(devices), ("core",))
    sharded = jax.jit(
        shard_map(_body, mesh=mesh,
                  in_specs=(PartitionSpec("core"),) * (n_params + n_outs),
                  out_specs=(PartitionSpec("core"),) * n_outs,
                  check_rep=False),
        donate_argnums=donate, keep_unused=True)

    param_order = in_names[:n_params]

    def run(in_maps):
        concat_in = [
            np.concatenate([np.asarray(in_maps[c][nm]) for c in range(NCORES)],
                           axis=0)
            for nm in param_order
        ]
        concat_zeros = [np.zeros((NCORES * z.shape[0], *z.shape[1:]), z.dtype)
                        for z in zero_outs]
        out_arrs = jax.block_until_ready(sharded(*concat_in, *concat_zeros))
        return [
            {nm: np.asarray(out_arrs[i]).reshape(NCORES, *out_avals[i].shape)[c]
             for i, nm in enumerate(out_names)}
            for c in range(NCORES)
        ]

    return {"run": run, "sharded": sharded, "param_order": param_order,
            "zero_outs": zero_outs, "out_names": out_names,
            "out_avals": out_avals, "mesh": mesh}


def _get_runner(w, reps=None, unroll=1):
    import os
    mm = os.environ.get("KERNEL_MM_DTYPE", "f16")
    if reps is None:
        reps = int(os.environ.get("KERNEL_REPS", "1"))
    key = (w, mm, reps, unroll)
    if key not in _prog_cache:
        nc, WROW = _build_program(w, mm=mm, reps=reps, unroll=unroll)
        _prog_cache[key] = (_make_runner(nc), WROW)
    return _prog_cache[key]


def kernel(x4, attention, W1, b1, W2, b2):
    import os
    Xp, attp, perms, w = _prepare(x4, attention)
    runner, WROW = _get_runner(w)
    mm = os.environ.get("KERNEL_MM_DTYPE", "f16")
    npdt = np.float16 if mm == "f16" else np.float32

    xpt = np.zeros((B, F, WROW), npdt)
    xpt[:, :, :N] = Xp.transpose(0, 2, 1)
    attp_pad = np.full((B, WROW), 1e9, np.float32)
    attp_pad[:, :N] = attp

    W1c = np.ascontiguousarray(W1, dtype=npdt)
    W2c = np.ascontiguousarray(W2, dtype=npdt)
    b1c = np.ascontiguousarray(b1, dtype=np.float32)
    b2c = np.ascontiguousarray(b2, dtype=np.float32)

    c_ones = np.ones((128, 1), npdt)
    c_onesrow = np.ones((1, N), npdt)
    c_estep = np.zeros((FCH, 95), npdt)
    c_estep[:, 47] = 1.0
    c_zeros = np.zeros((128, WROW), npdt)
    c_ident = np.eye(128, dtype=npdt)

    in_maps = []
    for c in range(NCORES):
        sl = slice(c * SPB, (c + 1) * SPB)
        in_maps.append({
            "xp": np.ascontiguousarray(Xp[sl].astype(npdt)),
            "xpt": np.ascontiguousarray(xpt[sl]),
            "attp": np.ascontiguousarray(attp_pad[sl]),
            "w1": W1c, "b1": b1c, "w2": W2c, "b2": b2c,
            "c_ones": c_ones, "c_onesrow": c_onesrow,
            "c_estep": c_estep, "c_zeros": c_zeros, "c_ident": c_ident,
        })

    results = runner["run"](in_maps)
    globals()["last_in_maps"] = in_maps
    globals()["last_runner"] = runner
    globals()["last_w"] = w

    inv = np.argsort(perms, axis=1)
    out = np.empty((B, N, F), np.float32)
    for c in range(NCORES):
        o = results[c]["outT"]  # [SPB, F, N]
        for s in range(SPB):
            bi = c * SPB + s
            out[bi] = o[s].T[inv[bi]]
    return out


# revision 32
# speedup vs baseline: 1.6601x; 1.3991x over previous
"""Trainium2 Bass kernel: batched ChebConv GNN with L1-distance adjacency.

Pipeline per sample (N=512 nodes, F=625 features):
  1. Sort nodes by attention (host). All pairs with |att_i-att_j| <= 0.05
     then lie within a rank band |i-j| <= w (w computed exactly on host).
  2. Banded pairwise L1 distances on device using the exact identity
     sum_f |a-b| = 2*sum_f max(a,b) - S_i - S_j  (S = row sums), computed
     with one fused DVE op per (offset-batch, feature-chunk) + a PE
     ones-matmul ("estep" staircase selector) for the cross-partition
     feature reduction.
  3. Threshold masks -> banded adjacency -> scattered to a DRAM matrix via
     diagonal-stride DMAs (identity on the diagonal).
  4. Degree-normalized ChebConv x2 as float16 PE matmuls in transposed
     layouts (the dinv column scaling is commuted through the weight
     matmuls so it is always a cheap per-partition row scaling).
Data parallel over batch: 16 samples, 8 cores, 2 samples/core.

dtype strategy (KERNEL_MM_DTYPE=f16 default): fp16 runs the DVE max ops
~2.8x faster than f32r on HW (612ns vs 1709ns per [125, 4x512] op),
halves DMA bytes, and keeps PE matmuls at ~190ns per 512-col instruction.
max(a,b) of fp16 inputs is exact; distance error comes only from rounding
x to fp16 (~3e-3 absolute on D ~ 176), so threshold misclassification is
negligible. PSUM accumulation stays fp32.

DMA: loads are split across both hardware DGE queues (SP via nc.sync and
Activation via nc.scalar) to double DMA issue parallelism.

The builder supports `unroll` bodies inside the `reps` hardware loop;
timing uses (unroll=2, reps=32) so consecutive kernel executions
double-buffer through the 2-deep tile pools (pool rotation happens per
traced body, not per loop iteration), overlapping one body's input
loads with the other body's compute - the steady-state throughput a
back-to-back caller would see.
"""

import numpy as np
from contextlib import ExitStack

B, N = 16, 512
F, FH = 625, 937
FCH, NFCH = 125, 5  # feature chunks: 5 x 125 = 625
NCORES = 8
SPB = B // NCORES  # samples per core
DIST_THRESH, ATT_THRESH = 180.0, 0.05
DCH = 48  # band offsets per PSUM group (psM tile base partition stays 0)

# FH row blocks (7x128 + 41)
FH_BLOCKS = [(o, min(128, FH - o)) for o in range(0, FH, 128)]

_prog_cache = {}


def _build_program(w, mm="f16", reps=1, unroll=1):
    """Build the SPMD Bass program for band half-width w.

    mm: dtype for matmul/DVE operands: "f16" (default), "f32r", "fp32".
    reps: hardware-loop iteration count (timing); unroll: kernel bodies
    per iteration (2 enables cross-execution double buffering).
    """
    import concourse.bass as bass
    import concourse.bacc as bacc
    import concourse.mybir as mybir
    import concourse.tile as tile

    dt = mybir.dt
    fp = dt.float32
    mdt = {"f16": dt.float16, "f32r": dt.float32r, "fp32": fp}[mm]
    AF = mybir.ActivationFunctionType
    OP = mybir.AluOpType
    AX = mybir.AxisListType
    AP = bass.AP

    padw = ((w + 7) // 8) * 8
    WROW = N + padw  # padded row width for xpt/attp/scratch

    nc = bacc.Bacc()
    xp_p = nc.declare_dram_parameter("xp", [SPB, N, F], mdt, isOutput=False)
    xpt_p = nc.declare_dram_parameter("xpt", [SPB, F, WROW], mdt, isOutput=False)
    attp_p = nc.declare_dram_parameter("attp", [SPB, WROW], fp, isOutput=False)
    w1_p = nc.declare_dram_parameter("w1", [2, F, FH], mdt, isOutput=False)
    b1_p = nc.declare_dram_parameter("b1", [FH], fp, isOutput=False)
    w2_p = nc.declare_dram_parameter("w2", [2, FH, F], mdt, isOutput=False)
    b2_p = nc.declare_dram_parameter("b2", [F], fp, isOutput=False)
    out_p = nc.declare_dram_parameter("outT", [SPB, F, N], fp, isOutput=True)
    ones_p = nc.declare_dram_parameter("c_ones", [128, 1], mdt, isOutput=False)
    onesrow_p = nc.declare_dram_parameter("c_onesrow", [1, N], mdt, isOutput=False)
    estep_p = nc.declare_dram_parameter("c_estep", [FCH, 95], mdt, isOutput=False)
    zeros_p = nc.declare_dram_parameter("c_zeros", [128, WROW], mdt, isOutput=False)
    ident_p = nc.declare_dram_parameter("c_ident", [128, 128], mdt, isOutput=False)
    ident32_p = nc.declare_dram_parameter("c_ident32", [DCH, DCH], fp, isOutput=False)

    # internal DRAM scratch, one set per (unroll body, sample slot)
    a_scr = [[nc.dram_tensor(f"a_scr{u}_{b}", [WROW * WROW], mdt)
              for b in range(SPB)] for u in range(unroll)]
    s_scr = [[nc.dram_tensor(f"s_scr{u}_{b}", [WROW], fp)
              for b in range(SPB)] for u in range(unroll)]
    d_scr = [[nc.dram_tensor(f"d_scr{u}_{b}", [N], fp)
              for b in range(SPB)] for u in range(unroll)]

    with tile.TileContext(nc) as tc, ExitStack() as ctx:
        cst = ctx.enter_context(tc.tile_pool(name="cst", bufs=1))
        xtp = ctx.enter_context(tc.tile_pool(name="xtp", bufs=min(2, unroll)))
        xpp = ctx.enter_context(tc.tile_pool(name="xpp", bufs=min(2, unroll)))
        mxp = ctx.enter_context(tc.tile_pool(name="mxp", bufs=2))
        bnd = ctx.enter_context(tc.tile_pool(name="bnd", bufs=1))
        amp = ctx.enter_context(tc.tile_pool(name="amp", bufs=min(2, unroll)))
        acp = ctx.enter_context(tc.tile_pool(name="acp", bufs=1))
        wsp = ctx.enter_context(tc.tile_pool(name="wsp", bufs=1))
        otp = ctx.enter_context(tc.tile_pool(name="otp", bufs=2))
        psp = ctx.enter_context(tc.tile_pool(name="psp", bufs=2, space="PSUM"))
        psb = ctx.enter_context(tc.tile_pool(name="psb", bufs=2, space="PSUM"))

        # ---- once-per-program init: adjacency scratch zeros + diagonal
        #      ones. Band cells are rewritten every execution; cells outside
        #      the band must stay zero, which zeroing once guarantees
        #      (scatters only ever touch band cells).
        onesrow = cst.tile([1, N], mdt, tag="onesrow")
        nc.sync.dma_start(onesrow[:], onesrow_p[:, :])
        for u in range(unroll):
            for b in range(SPB):
                ad = a_scr[u][b]
                for t in range(4):
                    nc.sync.dma_start(
                        AP(ad, t * 128 * WROW, [[1, 128 * WROW]]),
                        AP(zeros_p, 0, [[1, 128 * WROW]]))
                nc.sync.dma_start(AP(ad, 0, [[WROW + 1, N]]), onesrow[:])
            # s_scr pad rows [N, WROW) stay uninitialized; any value there
            # (even NaN) yields mask 0 because the attp pad is 1e9 and IEEE
            # comparisons with NaN are false.

        BD = 4  # band offsets per DVE instruction
        import os as _os
        phase = _os.environ.get("KERNEL_PHASE", "all")

        def body(u):
            """One full kernel execution (SPB samples)."""
            ascr, sscr, dscr = a_scr[u], s_scr[u], d_scr[u]

            # ---- inputs first (xt feeds the band phase = critical path)
            xt_all = []
            for b in range(SPB):
                xt = [xtp.tile([FCH, WROW], mdt, tag=f"xt{b}{c}",
                               name=f"xt{b}{c}") for c in range(NFCH)]
                for c in range(NFCH):
                    nc.sync.dma_start(xt[c][:],
                                      xpt_p[b, c * FCH:(c + 1) * FCH, :])
                xt_all.append(xt)
            do_cheb = phase in ("all", "cheb")
            do_band = phase in ("all", "band", "band_nosc")
            xn_all = []
            for b in range(SPB):
                xn = [xpp.tile([128, F], mdt, tag=f"xn{b}{t}",
                               name=f"xn{b}{t}") for t in range(4)]
                for t in range(4):
                    nc.scalar.dma_start(xn[t][:],
                                        xp_p[b, t * 128:(t + 1) * 128, :])
                xn_all.append(xn)

            # ---- constants + resident weights + biases (consumed later;
            #      issued after inputs so they don't delay the band phase)
            estep = cst.tile([FCH, 95], mdt, tag="estep", name="estep")
            nc.sync.dma_start(estep[:], estep_p[:, :])
            ident32 = cst.tile([DCH, DCH], fp, tag="ident32", name="ident32")
            nc.sync.dma_start(ident32[:], ident32_p[:, :])
            ident = cst.tile([128, 128], mdt, tag="ident", name="ident")
            nc.scalar.dma_start(ident[:], ident_p[:, :])
            if do_cheb:
                w1t = [[wsp.tile([FCH, FH], mdt, tag=f"w1t{k_}{c_}",
                                 name=f"w1t{k_}{c_}")
                        for c_ in range(NFCH)] for k_ in range(2)]
                for k_ in range(2):
                    for c_ in range(NFCH):
                        nc.sync.dma_start(w1t[k_][c_][:],
                                          w1_p[k_, c_ * FCH:(c_ + 1) * FCH, :])
                w2t = [[wsp.tile([128, F], mdt, tag=f"w2t{k_}{j_}",
                                 name=f"w2t{k_}{j_}")
                        for j_ in range(len(FH_BLOCKS))] for k_ in range(2)]
                for k_ in range(2):
                    for j_, (ko, kp) in enumerate(FH_BLOCKS):
                        nc.scalar.dma_start(w2t[k_][j_][:kp, :],
                                            w2_p[k_, ko:ko + kp, :])
                b1t = [wsp.tile([128, 1], fp, tag=f"b1t{j_}", name=f"b1t{j_}")
                       for j_ in range(len(FH_BLOCKS))]
                for j_, (mo, mp_) in enumerate(FH_BLOCKS):
                    nc.sync.dma_start(b1t[j_][:mp_, :], b1_p[mo:mo + mp_])
                b2t = [wsp.tile([FCH, 1], fp, tag=f"b2t{m_}", name=f"b2t{m_}")
                       for m_ in range(NFCH)]
                for m_ in range(NFCH):
                    nc.sync.dma_start(b2t[m_][:], b2_p[m_ * FCH:(m_ + 1) * FCH])

            # ---- phase 1: row sums via DVE free-axis reduce on xn blocks
            for b in range(SPB):
                for t in range(4):
                    srow = bnd.tile([128, 1], fp, tag="srow", name="srow")
                    nc.vector.tensor_reduce(srow[:], xn_all[b][t][:],
                                            axis=AX.X, op=OP.add)
                    nc.sync.dma_start(AP(sscr[b], t * 128, [[1, 128]]),
                                      srow[:])

            # ---- phases 2/3: bands and chebs, software-pipelined
            at_all = [None] * SPB

            def gen_band(b):
                ad, sd, dd = ascr[b], sscr[b], dscr[b]
                xt = xt_all[b]
                d0 = 1
                while do_band and d0 <= w:
                    dn = min(DCH, w - d0 + 1)
                    psM = psb.tile([dn, N], fp, tag="psM", name="psM")
                    nbatches = (dn + BD - 1) // BD
                    for bi in range(nbatches):
                        db0 = bi * BD
                        nb = min(BD, dn - db0)
                        for c in range(NFCH):
                            mxb = mxp.tile([FCH, BD * N], mdt, tag="mx",
                                           name="mxb")
                            base = xt[c][:, 0:N]
                            in0 = bass.AP(base.tensor, base.offset,
                                          [list(base.ap[0]), [0, nb], [1, N]])
                            in1 = bass.AP(base.tensor,
                                          base.offset + d0 + db0,
                                          [list(base.ap[0]), [1, nb], [1, N]])
                            nc.vector.tensor_tensor(
                                out=mxb[:, :nb * N], in0=in0, in1=in1,
                                op=OP.max)
                            for j in range(nb):
                                di = db0 + j
                                nc.tensor.matmul(
                                    psM[:, :],
                                    estep[:, 47 - di:47 - di + dn],
                                    mxb[:, j * N:(j + 1) * N],
                                    start=(di == 0 and c == 0),
                                    stop=(di == dn - 1 and c == NFCH - 1))
                            yield
                    # epilogue in node-major layout: per 128-node block,
                    # transpose psM to [128, dn] on the PE (tiny: 2cyc/row x
                    # dn), compute masks with per-partition scalars, and
                    # scatter the upper-triangle strip with CONTIGUOUS
                    # dn-element runs (addr(i, d) = i*(WROW+1) + d). The
                    # [dn, N] layout's diagonal scatters (stride WROW+1 on
                    # the inner axis = 21K scattered 2-byte writes) measured
                    # ~300 us/exec - this layout is ~100x fewer descriptors.
                    sM = bnd.tile([DCH, N], fp, tag="sM", name="sM")
                    nc.scalar.copy(sM[:dn, :], psM[:])
                    for t in range(4):
                        psT2 = psp.tile([128, DCH], fp, tag="psT2",
                                        name="psT2")
                        nc.tensor.transpose(
                            psT2[:, :dn], sM[:dn, t * 128:(t + 1) * 128],
                            ident32[:dn, :dn])
                        Sb = bnd.tile([128, 1], fp, tag="Sb", name="Sb")
                        nc.sync.dma_start(Sb[:], AP(sd, t * 128, [[1, 128]]))
                        Ss = bnd.tile([128, DCH], fp, tag="Ss", name="Ss")
                        nc.sync.dma_start(
                            Ss[:, :dn],
                            AP(sd, t * 128 + d0, [[1, 128], [1, dn]]))
                        Ab = bnd.tile([128, 1], fp, tag="Ab", name="Ab")
                        nc.sync.dma_start(
                            Ab[:], AP(attp_p, b * WROW + t * 128, [[1, 128]]))
                        As = bnd.tile([128, DCH], fp, tag="As", name="As")
                        nc.sync.dma_start(
                            As[:, :dn],
                            AP(attp_p, b * WROW + t * 128 + d0,
                               [[1, 128], [1, dn]]))
                        thr = bnd.tile([128, DCH], fp, tag="thr", name="thr")
                        nc.vector.tensor_scalar(thr[:, :dn], Ss[:, :dn],
                                                Sb[:], DIST_THRESH,
                                                op0=OP.add, op1=OP.add)
                        md = bnd.tile([128, DCH], fp, tag="md", name="md")
                        nc.vector.scalar_tensor_tensor(
                            out=md[:, :dn], in0=psT2[:, :dn], scalar=2.0,
                            in1=thr[:, :dn], op0=OP.mult, op1=OP.is_le)
                        ma = bnd.tile([128, DCH], fp, tag="ma", name="ma")
                        nc.vector.tensor_scalar(ma[:, :dn], As[:, :dn],
                                                Ab[:], ATT_THRESH,
                                                op0=OP.subtract, op1=OP.is_le)
                        abndT = bnd.tile([128, DCH], mdt, tag="abndT",
                                         name="abndT")
                        nc.vector.tensor_mul(abndT[:, :dn], md[:, :dn],
                                             ma[:, :dn])
                        if phase != "band_nosc":
                            nc.scalar.dma_start(
                                AP(ad, t * 128 * (WROW + 1) + d0,
                                   [[WROW + 1, 128], [1, dn]]),
                                abndT[:, :dn])
                    d0 += dn
                    yield

                if phase == "band_nosc":
                    return
                # a_scr holds only the strictly-upper band (+0.5 on the
                # diagonal). Row block t of (A+I) = upper-rows + transposed
                # upper-column-block; the transposed parts are PE transposes
                # of the already-loaded at tiles (only band-adjacent block
                # pairs |t-t'| <= 1 are nonzero for 2w < 128), and the two
                # 0.5 diagonal halves sum to the identity's 1.
                at = [amp.tile([128, N], mdt, tag=f"at{b}{t}",
                               name=f"at{b}{t}") for t in range(4)]
                for t in range(4):
                    nc.scalar.dma_start(
                        at[t][:], AP(ad, t * 128 * WROW, [[WROW, 128], [1, N]]))
                nblk = (w + 127) // 128  # how far the band spills over blocks
                # only tp <= t contributes: U is upper-triangular, so the
                # transposed block U[rows tp, cols t]^T is zero for tp > t
                pairs = [(t, tp) for t in range(4)
                         for tp in range(max(0, t - nblk), t + 1)]
                hbxs = {}
                for (t, tp) in pairs:
                    psX = psp.tile([128, 128], mdt, tag="psT", name="psX")
                    nc.tensor.transpose(
                        psX[:], at[tp][:, t * 128:(t + 1) * 128], ident[:, :])
                    hbx = bnd.tile([128, 128], mdt, tag=f"hbx{t}{tp}",
                                   name=f"hbx{t}{tp}")
                    nc.scalar.copy(hbx[:], psX[:])
                    hbxs[(t, tp)] = hbx
                for (t, tp) in pairs:
                    nc.vector.tensor_add(
                        at[t][:, tp * 128:(tp + 1) * 128],
                        at[t][:, tp * 128:(tp + 1) * 128], hbxs[(t, tp)][:])
                for t in range(4):
                    deg = bnd.tile([128, 1], fp, tag="deg", name="deg")
                    nc.vector.tensor_reduce(deg[:], at[t][:], axis=AX.X,
                                            op=OP.add)
                    dv = bnd.tile([128, 1], fp, tag="dv", name="dv")
                    nc.vector.reciprocal(dv[:], deg[:])
                    nc.sync.dma_start(AP(dd, t * 128, [[1, 128]]), dv[:])
                dinvB = amp.tile([128, N], fp, tag=f"dinvB{b}",
                                 name=f"dinvB{b}")
                nc.sync.dma_start(dinvB[:], AP(dd, 0, [[0, 128], [1, N]]))
                # at' = (A+I) diag(1/deg): both Cheb terms use it directly
                for t in range(4):
                    nc.vector.tensor_mul(at[t][:], at[t][:], dinvB[:])
                at_all[b] = at
                yield

            def gen_cheb(b):
                xt, at, xn = xt_all[b], at_all[b], xn_all[b]

                zt = [acp.tile([FCH, N], mdt, tag=f"zt{m}", name=f"zt{m}")
                      for m in range(NFCH)]
                for m in range(NFCH):
                    psZ = psp.tile([FCH, N], fp, tag="mm", name="psZ")
                    for t in range(4):
                        nc.tensor.matmul(psZ[:],
                                         xn[t][:, m * FCH:(m + 1) * FCH],
                                         at[t][:], start=(t == 0),
                                         stop=(t == 3))
                    nc.scalar.copy(zt[m][:], psZ[:])
                    yield

                ht = [acp.tile([128, N], mdt, tag=f"ht{k}", name=f"ht{k}")
                      for k in range(len(FH_BLOCKS))]
                for k, (mo, mp_) in enumerate(FH_BLOCKS):
                    psH = psp.tile([128, N], fp, tag="mm", name="psH")
                    for c in range(NFCH):
                        nc.tensor.matmul(psH[:mp_, :],
                                         w1t[0][c][:, mo:mo + mp_],
                                         xt[c][:, :N], start=(c == 0),
                                         stop=False)
                    for c in range(NFCH):
                        nc.tensor.matmul(psH[:mp_, :],
                                         w1t[1][c][:, mo:mo + mp_],
                                         zt[c][:], start=False,
                                         stop=(c == NFCH - 1))
                    nc.scalar.activation(ht[k][:mp_, :], psH[:mp_, :],
                                         AF.Relu, bias=b1t[k][:mp_, :],
                                         scale=1.0)
                    yield

                qt = [acp.tile([128, N], mdt, tag=f"qt{k}", name=f"qt{k}")
                      for k in range(len(FH_BLOCKS))]
                for k, (mo, mp_) in enumerate(FH_BLOCKS):
                    psQ = psp.tile([128, N], fp, tag="mm", name="psQ")
                    for t in range(4):
                        psT = psp.tile([128, 128], mdt, tag="psT", name="psT")
                        nc.tensor.transpose(
                            psT[:, :mp_],
                            ht[k][:mp_, t * 128:(t + 1) * 128],
                            ident[:mp_, :mp_])
                        hb = bnd.tile([128, 128], mdt, tag="hb", bufs=3,
                                      name="hb")
                        nc.scalar.copy(hb[:, :mp_], psT[:, :mp_])
                        nc.tensor.matmul(psQ[:mp_, :], hb[:, :mp_], at[t][:],
                                         start=(t == 0), stop=(t == 3))
                    nc.scalar.copy(qt[k][:mp_, :], psQ[:mp_, :])
                    yield

                for m in range(NFCH):
                    psO = psp.tile([FCH, N], fp, tag="mm", name="psO")
                    for k, (ko, kp) in enumerate(FH_BLOCKS):
                        nc.tensor.matmul(psO[:],
                                         w2t[0][k][:kp, m * FCH:(m + 1) * FCH],
                                         ht[k][:kp, :], start=(k == 0),
                                         stop=False)
                    for k, (ko, kp) in enumerate(FH_BLOCKS):
                        nc.tensor.matmul(psO[:],
                                         w2t[1][k][:kp, m * FCH:(m + 1) * FCH],
                                         qt[k][:kp, :], start=False,
                                         stop=(k == len(FH_BLOCKS) - 1))
                    ot = otp.tile([FCH, N], fp, tag="ot", name="ot")
                    nc.scalar.activation(ot[:], psO[:], AF.Relu,
                                         bias=b2t[m][:], scale=1.0)
                    nc.scalar.dma_start(out_p[b, m * FCH:(m + 1) * FCH, :],
                                        ot[:])
                    yield

            # band0 fully; then interleave band1 units with cheb0 units so
            # ChebConv-0 matmuls fill PE slack while DVE paces band1;
            # cheb1 last
            for _ in gen_band(0):
                pass
            if do_cheb:
                g_band1, g_cheb0 = gen_band(1), gen_cheb(0)
                done_b = done_c = False
                while not (done_b and done_c):
                    for _ in range(2):
                        if not done_b:
                            done_b = next(g_band1, StopIteration) is StopIteration
                    if not done_c:
                        done_c = next(g_cheb0, StopIteration) is StopIteration
                for _ in gen_cheb(1):
                    pass
            else:
                for _ in gen_band(1):
                    pass

        rep_cm = tc.For_i(0, reps, 1) if reps > 1 else None
        if rep_cm is not None:
            rep_cm.__enter__()
        for u in range(unroll):
            body(u)
        if rep_cm is not None:
            rep_cm.__exit__(None, None, None)

    if not nc.is_finalized():
        nc.finalize()
    return nc, WROW


def _prepare(x4, attention):
    """Host prep: flatten, sort by attention, compute band width, pad."""
    X = np.ascontiguousarray(x4.reshape(B, N, F), dtype=np.float32)
    att = np.ascontiguousarray(attention[:, :, 0, 0], dtype=np.float32)
    perms = np.argsort(att, axis=1, kind="stable")
    attp = np.take_along_axis(att, perms, axis=1)
    a64 = attp.astype(np.float64)
    w = 1
    for bi in range(B):
        for d in range(1, N):
            if np.min(a64[bi, d:] - a64[bi, :-d]) <= ATT_THRESH + 1e-6:
                w = max(w, d)
            else:
                break  # windows only widen with d
    w = min(w, N - 1)
    Xp = np.take_along_axis(X, perms[:, :, None], axis=1)
    return Xp, attp, perms, w


def _make_runner(nc):
    """Compile the Bass program into a reusable 8-core sharded jax callable.

    Mirrors concourse.bass2jax.run_bass_via_pjrt's multi-core branch, but
    returns the compiled callable so repeated executions can be timed.
    """
    import jax
    from jax.sharding import Mesh, PartitionSpec
    from jax.experimental.shard_map import shard_map
    from concourse import bass2jax, mybir

    bass2jax.install_neuronx_cc_hook()

    in_names, out_names, out_avals, zero_outs = [], [], [], []
    partition_name = (nc.partition_id_tensor.name
                      if nc.partition_id_tensor else None)
    for alloc in nc.m.functions[0].allocations:
        if not isinstance(alloc, mybir.MemoryLocationSet):
            continue
        name = alloc.memorylocations[0].name
        if alloc.kind == "ExternalInput":
            if name != partition_name:
                in_names.append(name)
        elif alloc.kind == "ExternalOutput":
            shape = tuple(alloc.tensor_shape)
            dtype = mybir.dt.np(alloc.dtype)
            out_names.append(name)
            out_avals.append(jax.core.ShapedArray(shape, dtype))
            zero_outs.append(np.zeros(shape, dtype))
    n_params = len(in_names)
    n_outs = len(out_avals)
    in_names = in_names + out_names
    if partition_name is not None:
        in_names.append(partition_name)
    donate = tuple(range(n_params, n_params + n_outs))

    def _body(*args):
        operands = list(args)
        if partition_name is not None:
            operands.append(bass2jax.partition_id_tensor())
        outs = bass2jax._bass_exec_p.bind(
            *operands,
            out_avals=tuple(out_avals),
            in_names=tuple(in_names),
            out_names=tuple(out_names),
            lowering_input_output_aliases=(),
            sim_require_finite=True,
            sim_require_nnan=True,
            nc=nc,
        )
        return tuple(outs)

    devices = jax.devices()[:NCORES]
    mesh = Mesh(np.asarray(devices), ("core",))
    sharded = jax.jit(
        shard_map(_body, mesh=mesh,
                  in_specs=(PartitionSpec("core"),) * (n_params + n_outs),
                  out_specs=(PartitionSpec("core"),) * n_outs,
                  check_rep=False),
        donate_argnums=donate, keep_unused=True)

    param_order = in_names[:n_params]

    def run(in_maps):
        concat_in = [
            np.concatenate([np.asarray(in_maps[c][nm]) for c in range(NCORES)],
                           axis=0)
            for nm in param_order
        ]
        concat_zeros = [np.zeros((NCORES * z.shape[0], *z.shape[1:]), z.dtype)
                        for z in zero_outs]
        out_arrs = jax.block_until_ready(sharded(*concat_in, *concat_zeros))
        return [
            {nm: np.asarray(out_arrs[i]).reshape(NCORES, *out_avals[i].shape)[c]
             for i, nm in enumerate(out_names)}
            for c in range(NCORES)
        ]

    return {"run": run, "sharded": sharded, "param_order": param_order,
            "zero_outs": zero_outs, "out_names": out_names,
            "out_avals": out_avals, "mesh": mesh}


def _get_runner(w, reps=None, unroll=1):
    import os
    mm = os.environ.get("KERNEL_MM_DTYPE", "f16")
    if reps is None:
        reps = int(os.environ.get("KERNEL_REPS", "1"))
    key = (w, mm, reps, unroll)
    if key not in _prog_cache:
        nc, WROW = _build_program(w, mm=mm, reps=reps, unroll=unroll)
        _prog_cache[key] = (_make_runner(nc), WROW)
    return _prog_cache[key]


def kernel(x4, attention, W1, b1, W2, b2):
    import os
    Xp, attp, perms, w = _prepare(x4, attention)
    runner, WROW = _get_runner(w)
    mm = os.environ.get("KERNEL_MM_DTYPE", "f16")
    npdt = np.float16 if mm == "f16" else np.float32

    xpt = np.zeros((B, F, WROW), npdt)
    xpt[:, :, :N] = Xp.transpose(0, 2, 1)
    attp_pad = np.full((B, WROW), 1e9, np.float32)
    attp_pad[:, :N] = attp

    W1c = np.ascontiguousarray(W1, dtype=npdt)
    W2c = np.ascontiguousarray(W2, dtype=npdt)
    b1c = np.ascontiguousarray(b1, dtype=np.float32)
    b2c = np.ascontiguousarray(b2, dtype=np.float32)

    c_ones = np.ones((128, 1), npdt)
    # 0.5 on the a_scr diagonal: row-load and transposed-column-load halves
    # sum to the identity's 1
    c_onesrow = np.full((1, N), 0.5, npdt)
    c_estep = np.zeros((FCH, 95), npdt)
    c_estep[:, 47] = 1.0
    c_zeros = np.zeros((128, WROW), npdt)
    c_ident = np.eye(128, dtype=npdt)
    c_ident32 = np.eye(DCH, dtype=np.float32)

    in_maps = []
    for c in range(NCORES):
        sl = slice(c * SPB, (c + 1) * SPB)
        in_maps.append({
            "xp": np.ascontiguousarray(Xp[sl].astype(npdt)),
            "xpt": np.ascontiguousarray(xpt[sl]),
            "attp": np.ascontiguousarray(attp_pad[sl]),
            "w1": W1c, "b1": b1c, "w2": W2c, "b2": b2c,
            "c_ones": c_ones, "c_onesrow": c_onesrow,
            "c_estep": c_estep, "c_zeros": c_zeros, "c_ident": c_ident,
            "c_ident32": c_ident32,
        })

    results = runner["run"](in_maps)
    globals()["last_in_maps"] = in_maps
    globals()["last_runner"] = runner
    globals()["last_w"] = w

    inv = np.argsort(perms, axis=1)
    out = np.empty((B, N, F), np.float32)
    for c in range(NCORES):
        o = results[c]["outT"]  # [SPB, F, N]
        for s in range(SPB):
            bi = c * SPB + s
            out[bi] = o[s].T[inv[bi]]
    return out
